# revision 1
# baseline (speedup 1.0000x reference)
"""Chunked cross-attention TRN2 kernel (8 NeuronCores, SPMD).

Problem (hardcoded): B=4, S=2048, HIDDEN=1024, heads=16, head_dim=64,
chunks C=32 x chunk_len 64, neighbors N=2 x L=128 (256 keys per chunk).

Sharding: the B*C = 128 (batch, chunk) pairs are split evenly across the 8
cores (16 pairs each). Each core projects Q/K/V for its pairs, runs the
chunk-local softmax attention, and writes its pairs' outputs. Weights are
replicated per core. No collectives needed.

Numerics: projection matmuls run as float32r (TF32-class, full PE rate at
free-dim >= 256); the attention matmuls (scores/transpose/AV) run in bf16,
which legalizes tile_position head-packing. Accumulation is always fp32 in
PSUM. Softmax runs without max-subtraction (shift-invariant, and
|scores/8| is small for randn-scale inputs so exp cannot overflow); the
exp is computed by ScalarE with a fused per-row sum, and the 1/sum is
applied on the AV result during the PSUM->SBUF copyback.

Layouts (host-prepared so the device never transposes activations):
  q_t   [1024, 16*64]   f32r  shifted/padded query, hidden-major
  kv_t  [1024, 16*256]  f32r  kv rows, hidden-major
  wq_t  [8*128*8*128]   f32r  W_q.T prepacked per m-tile [mo, p, ko, m]
  wk_t, wv_t [1024,1024] f32r W.T (contraction-major)
  bq_t/bk_t [128, 8]    f32   bias striped per m-subtile
  bv_r  [128, 1024]     f32   V bias replicated across partitions
Device out [16, 128, 512] f32: partitions = 2 heads x 64 rows (head-pair
packing via tile_position quadrants), free = head_pair*64 + d. The host
unpacks this and applies the chunked-attention output shift.
"""

import numpy as np

B, S, HID = 4, 2048, 1024
C, NNB, L = 32, 2, 128
CHUNK = 64
NHEADS, HEAD = 16, 64
NCORES = 8
NPAIRS = B * C                 # 128
PER_CORE = NPAIRS // NCORES    # 16
QBLK = 4                       # pairs per Q-projection block (rows = 256)
GRP = 2                        # pairs per K/V-projection group (rows = 512)
P = 128
KSUB = HID // P                # 8
MSUB = HID // P                # 8
JTOT = NNB * L                 # 256 keys per chunk
SCALE = 0.125                  # 1/sqrt(HEAD)

_CACHE = {}


def _build():
    from contextlib import ExitStack

    import concourse.bass as bass
    import concourse.mybir as mybir
    import concourse.tile as tile
    from concourse import bacc
    from concourse.masks import make_identity

    f32 = mybir.dt.float32
    f32r = mybir.dt.float32r
    bf16 = mybir.dt.bfloat16

    nc = bacc.Bacc("TRN2", target_bir_lowering=False, debug=False,
                   num_devices=NCORES)

    q_t = nc.dram_tensor("q_t", [HID, PER_CORE * CHUNK], f32r, kind="ExternalInput")
    kv_t = nc.dram_tensor("kv_t", [HID, PER_CORE * JTOT], f32r, kind="ExternalInput")
    wq_t = nc.dram_tensor("wq_t", [MSUB * P * KSUB * P], f32r, kind="ExternalInput")
    wk_t = nc.dram_tensor("wk_t", [HID, HID], f32r, kind="ExternalInput")
    wv_t = nc.dram_tensor("wv_t", [HID, HID], f32r, kind="ExternalInput")
    bq_t = nc.dram_tensor("bq_t", [P, MSUB], f32, kind="ExternalInput")
    bk_t = nc.dram_tensor("bk_t", [P, MSUB], f32, kind="ExternalInput")
    bv_r = nc.dram_tensor("bv_r", [P, HID], f32, kind="ExternalInput")
    out = nc.dram_tensor("out", [PER_CORE, P, NHEADS * HEAD // 2], f32,
                         kind="ExternalOutput")

    q_td = q_t[:].rearrange("(ko p) r -> p ko r", p=P)
    kv_td = kv_t[:].rearrange("(ko p) r -> p ko r", p=P)
    wq_packed = wq_t[:].rearrange("(mo p ko m) -> mo p ko m",
                                  mo=MSUB, p=P, ko=KSUB)
    wk_td = wk_t[:].rearrange("(ko p) m -> p ko m", p=P)
    wv_td = wv_t[:].rearrange("(ko p) m -> p ko m", p=P)

    with tile.TileContext(nc) as tc:
        with ExitStack() as ctx:
            wpool = ctx.enter_context(tc.tile_pool(name="weights", bufs=1))
            qtp = ctx.enter_context(tc.tile_pool(name="qt", bufs=1))
            qpp = ctx.enter_context(tc.tile_pool(name="qproj", bufs=2))
            kvp = ctx.enter_context(tc.tile_pool(name="kvt", bufs=2))
            kpp = ctx.enter_context(tc.tile_pool(name="kproj", bufs=3))
            vpp = ctx.enter_context(tc.tile_pool(name="vproj", bufs=3))
            sfp = ctx.enter_context(tc.tile_pool(name="soft", bufs=3))
            smalls = ctx.enter_context(tc.tile_pool(name="smalls", bufs=6))
            outp = ctx.enter_context(tc.tile_pool(name="outsb", bufs=2))
            ps_pj = ctx.enter_context(tc.tile_pool(name="ps_pj", bufs=3, space="PSUM"))
            ps_sc = ctx.enter_context(tc.tile_pool(name="ps_sc", bufs=2, space="PSUM"))
            ps_tr = ctx.enter_context(tc.tile_pool(name="ps_tr", bufs=1, space="PSUM"))
            ps_av = ctx.enter_context(tc.tile_pool(name="ps_av", bufs=2, space="PSUM"))

            # --- resident constants. DMA order minimizes bytes before the
            # first matmul (wq m-tile 0 + q block 0) and before the first
            # K-projection (wk + first kv tiles ahead of wv). ---
            bq_sb = wpool.tile([P, MSUB], f32)
            nc.sync.dma_start(bq_sb[:], bq_t[:])
            wq_ts = []
            for mo in range(MSUB):
                w = wpool.tile([P, KSUB, P], f32r, name=f"wq{mo}")
                nc.sync.dma_start(w[:], wq_packed[mo])
                wq_ts.append(w)
                if mo == 0:
                    qt_first = qtp.tile([P, KSUB, QBLK * CHUNK], f32r,
                                        tag="qt", name="qt_first")
                    nc.sync.dma_start(qt_first[:],
                                      q_td[:, :, bass.ts(0, QBLK * CHUNK)])
            bk_sb = wpool.tile([P, MSUB], f32)
            nc.sync.dma_start(bk_sb[:], bk_t[:])
            wk_sb = wpool.tile([P, KSUB, HID], f32r)
            nc.sync.dma_start(wk_sb[:], wk_td)
            # block 0's kv tiles load before wv so the K projection can
            # start as soon as the Q projection drains
            kvt_first = []
            for g2 in range(QBLK // GRP):
                kvt_sb = kvp.tile([P, KSUB, GRP * JTOT], f32r,
                                  tag="kvt", name=f"kvt_first{g2}")
                nc.sync.dma_start(
                    kvt_sb[:], kv_td[:, :, bass.ds(g2 * GRP * JTOT, GRP * JTOT)])
                kvt_first.append(kvt_sb)
            wv_sb = wpool.tile([P, KSUB, HID], f32r)
            nc.sync.dma_start(wv_sb[:], wv_td)
            bv_sb = wpool.tile([P, HID], f32)
            nc.sync.dma_start(bv_sb[:], bv_r[:])
            ident32 = wpool.tile([P, P], f32)
            make_identity(nc, ident32[:])
            ident = wpool.tile([P, P], bf16)
            nc.vector.tensor_copy(ident[:], ident32[:])

            Exp = mybir.ActivationFunctionType.Exp
            Ident = mybir.ActivationFunctionType.Identity

            for blk in range(PER_CORE // QBLK):
                # ---- Q projection for this block (rows = QBLK*64 = 256) ----
                if blk == 0:
                    qt_sb = qt_first
                else:
                    qt_sb = qtp.tile([P, KSUB, QBLK * CHUNK], f32r, tag="qt",
                                     name="qt_sb")
                    nc.sync.dma_start(qt_sb[:],
                                      q_td[:, :, bass.ts(blk, QBLK * CHUNK)])
                qp_sb = qpp.tile([P, MSUB, QBLK * CHUNK], bf16)
                for mo in range(MSUB):
                    pt = ps_pj.tile([P, 512], f32, tag="ps_pj", name="pt")
                    pt = pt[:, : QBLK * CHUNK]
                    for k in range(KSUB):
                        nc.tensor.matmul(
                            pt[:],
                            wq_ts[mo][:, k, :],
                            qt_sb[:, k, :],
                            start=(k == 0),
                            stop=(k == KSUB - 1),
                        )
                    nc.scalar.activation(qp_sb[:, mo, :], pt[:], Ident,
                                         bias=bq_sb[:, mo, None])

                # ---- K/V projections, 2 groups of 512 kv rows each ----
                if blk == 0:
                    kvt_ts = kvt_first
                else:
                    kvt_ts = []
                    for g2 in range(QBLK // GRP):
                        kvt_sb = kvp.tile([P, KSUB, GRP * JTOT], f32r,
                                          tag="kvt", name=f"kvt{g2}")
                        nc.sync.dma_start(
                            kvt_sb[:],
                            kv_td[:, :, bass.ds((blk * QBLK + g2 * GRP) * JTOT,
                                                GRP * JTOT)])
                        kvt_ts.append(kvt_sb)

                kp_ts = [kpp.tile([P, MSUB, GRP * JTOT], bf16, tag="kp",
                                  name=f"kp{g2}")
                         for g2 in range(QBLK // GRP)]
                for g2 in range(QBLK // GRP):
                    for mo in range(MSUB):
                        pt = ps_pj.tile([P, 512], f32, tag="ps_pj", name="pt")
                        for k in range(KSUB):
                            nc.tensor.matmul(
                                pt[:],
                                wk_sb[:, k, bass.ts(mo, P)],
                                kvt_ts[g2][:, k, :],
                                start=(k == 0),
                                stop=(k == KSUB - 1),
                            )
                        nc.scalar.activation(kp_ts[g2][:, mo, :], pt[:],
                                             Ident, bias=bk_sb[:, mo, None])

                vp_ts = [vpp.tile([P, 2 * GRP, HID], bf16, tag="vp",
                                  name=f"vp{g2}")
                         for g2 in range(QBLK // GRP)]
                for g2 in range(QBLK // GRP):
                    for rt in range(2 * GRP):
                        for nt in range(2):
                            pt = ps_pj.tile([P, 512], f32, tag="ps_pj",
                                            name="pt")
                            for k in range(KSUB):
                                nc.tensor.matmul(
                                    pt[:],
                                    kvt_ts[g2][:, k, bass.ts(rt, P)],
                                    wv_sb[:, k, bass.ts(nt, 512)],
                                    start=(k == 0),
                                    stop=(k == KSUB - 1),
                                )
                            nc.vector.tensor_tensor(
                                vp_ts[g2][:, rt, bass.ts(nt, 512)], pt[:],
                                bv_sb[:, bass.ts(nt, 512)],
                                mybir.AluOpType.add,
                            )

                # ---- attention: two heads packed per 128 partitions ----
                for pi in range(QBLK):
                    g2, g = pi // GRP, pi % GRP
                    kp_sb, vp_sb = kp_ts[g2], vp_ts[g2]
                    gp = blk * QBLK + pi
                    out_sb = outp.tile([P, NHEADS * HEAD // 2], f32,
                                       tag="out_sb")
                    for hp in range(NHEADS // 2):
                        ps_s = ps_sc.tile([P, JTOT], f32, tag="ps_s")
                        nc.tensor.matmul(
                            ps_s[0:64, :],
                            qp_sb[0:64, hp, bass.ts(pi, CHUNK)],
                            kp_sb[0:64, hp, bass.ts(g, JTOT)],
                            start=True, stop=True, tile_position=(0, 0),
                        )
                        nc.tensor.matmul(
                            ps_s[64:128, :],
                            qp_sb[64:128, hp, bass.ts(pi, CHUNK)],
                            kp_sb[64:128, hp, bass.ts(g, JTOT)],
                            start=True, stop=True, tile_position=(64, 64),
                        )
                        attn = sfp.tile([P, JTOT], bf16, tag="attn")
                        rsum = smalls.tile([P, 1], f32, tag="rsum")
                        nc.scalar.activation(attn[:], ps_s[:], Exp,
                                             scale=SCALE, accum_out=rsum[:])
                        recip = smalls.tile([P, 1], f32, tag="recip")
                        nc.vector.reciprocal(recip[:], rsum[:])

                        ps_t = ps_tr.tile([P, 2, P], bf16, tag="ps_t")
                        for jh in range(2):
                            nc.tensor.transpose(
                                ps_t[:, jh, :], attn[:, bass.ts(jh, P)],
                                ident)
                        at_t = sfp.tile([P, 2, P], bf16, tag="at_t")
                        nc.vector.tensor_copy(at_t[:], ps_t[:])

                        ps_o = ps_av.tile([P, HEAD], f32, tag="ps_o")
                        for jh in range(2):
                            nc.tensor.matmul(
                                ps_o[0:64, :],
                                at_t[:, jh, 0:64],
                                vp_sb[:, 2 * g + jh,
                                      bass.ds(2 * hp * HEAD, HEAD)],
                                start=(jh == 0), stop=(jh == 1),
                                tile_position=(0, 0),
                            )
                            nc.tensor.matmul(
                                ps_o[64:128, :],
                                at_t[:, jh, 64:128],
                                vp_sb[:, 2 * g + jh,
                                      bass.ds((2 * hp + 1) * HEAD, HEAD)],
                                start=(jh == 0), stop=(jh == 1),
                                tile_position=(0, 64),
                            )
                        nc.vector.tensor_scalar_mul(
                            out_sb[:, bass.ts(hp, HEAD)], ps_o[:], recip[:])

                    nc.sync.dma_start(out[gp], out_sb[:])

    nc.finalize()
    return nc


def _prepare_inputs(query, kv, Wq, bq, Wk, bk, Wv, bv):
    """Build the 8 per-core input maps (host-side shard + layout + cast)."""
    import ml_dtypes

    f32 = np.float32
    bf = ml_dtypes.bfloat16
    query = np.asarray(query, dtype=f32)
    kv = np.asarray(kv, dtype=f32)

    # shift right by CHUNK-1, pad to C*CHUNK rows
    q_shift = np.zeros((B, C * CHUNK, HID), dtype=f32)
    q_shift[:, : S - (CHUNK - 1)] = query[:, CHUNK - 1:]
    q_pairs = q_shift.reshape(B * C, CHUNK, HID)
    kv_pairs = kv.reshape(B * C, JTOT, HID)

    wq_tt = np.asarray(Wq, dtype=f32).T  # [h, m]
    wq_t = np.ascontiguousarray(
        wq_tt.reshape(KSUB, P, MSUB, P).transpose(2, 1, 0, 3)
    ).reshape(-1)
    wk_t = np.ascontiguousarray(np.asarray(Wk, dtype=f32).T)
    wv_t = np.ascontiguousarray(np.asarray(Wv, dtype=f32).T)
    bq_t = np.ascontiguousarray(np.asarray(bq, dtype=f32).reshape(MSUB, P).T)
    bk_t = np.ascontiguousarray(np.asarray(bk, dtype=f32).reshape(MSUB, P).T)
    bv_rep = np.ascontiguousarray(
        np.broadcast_to(np.asarray(bv, dtype=f32), (P, HID)))

    in_maps = []
    for ci in range(NCORES):
        sel = slice(ci * PER_CORE, (ci + 1) * PER_CORE)
        q_core = q_pairs[sel].reshape(PER_CORE * CHUNK, HID)
        kv_core = kv_pairs[sel].reshape(PER_CORE * JTOT, HID)
        in_maps.append({
            "q_t": np.ascontiguousarray(q_core.T),
            "kv_t": np.ascontiguousarray(kv_core.T),
            "wq_t": wq_t,
            "wk_t": wk_t,
            "wv_t": wv_t,
            "bq_t": bq_t,
            "bk_t": bk_t,
            "bv_r": bv_rep,
        })
    return in_maps


def _unpack_output(results):
    """results: list of 8 dicts with 'out' [16, 128, 512] -> full (B,S,HID)."""
    h = np.empty((NPAIRS, CHUNK, HID), dtype=np.float32)
    for ci in range(NCORES):
        arr = results[ci]["out"]
        a = arr.reshape(PER_CORE, 2, CHUNK, NHEADS // 2, HEAD)
        a = a.transpose(0, 2, 3, 1, 4).reshape(PER_CORE, CHUNK, HID)
        h[ci * PER_CORE:(ci + 1) * PER_CORE] = a
    h = h.reshape(B, C * CHUNK, HID)
    outp = np.zeros((B, S, HID), dtype=np.float32)
    outp[:, CHUNK - 1:] = h[:, : S - (CHUNK - 1)]
    return outp


def kernel(query, kv, Wq, bq, Wk, bk, Wv, bv):
    from concourse.bass_utils import run_bass_kernel_spmd

    if "nc" not in _CACHE:
        _CACHE["nc"] = _build()
    nc = _CACHE["nc"]

    in_maps = _prepare_inputs(query, kv, Wq, bq, Wk, bk, Wv, bv)
    res = run_bass_kernel_spmd(nc, in_maps, list(range(NCORES)))
    return _unpack_output(res.results)



# revision 3
# speedup vs baseline: 1.0312x; 1.0312x over previous
"""Chunked cross-attention TRN2 kernel (8 NeuronCores, SPMD).

Problem (hardcoded): B=4, S=2048, HIDDEN=1024, heads=16, head_dim=64,
chunks C=32 x chunk_len 64, neighbors N=2 x L=128 (256 keys per chunk).

Sharding: the B*C = 128 (batch, chunk) pairs are split evenly across the 8
cores (16 pairs each). Each core projects Q/K/V for its pairs, runs the
chunk-local softmax attention, and writes its pairs' outputs. Weights are
replicated per core. No collectives needed.

All matmul operands are bf16 (stationary tiles are 128-column so FWL is
active; fp32 accumulation in PSUM). The attention avoids PE transposes
entirely: scores are computed pre-transposed as out[keys, queries] with a
block-diagonal [dA|dB]x[qA|qB] moving operand built during the Q
projection (zeros kill the cross-head terms), packing two heads per
matmul. The AV matmul consumes the [keys, q] attention weights directly;
a ones-column appended to V yields the softmax normalizer in the same
matmul, and the 1/sum scale is applied by ScalarE during PSUM copyback.

Per-block schedule interleaves the V projection with attention per pair
so the ScalarE exp runs under PE matmuls of the next pair.
"""

import numpy as np

B, S, HID = 4, 2048, 1024
C, NNB, L = 32, 2, 128
CHUNK = 64
NHEADS, HEAD = 16, 64
NCORES = 8
NPAIRS = B * C                 # 128
PER_CORE = NPAIRS // NCORES    # 16
QBLK = 4                       # pairs per block (256 q rows, 1024 kv rows)
NBLK = PER_CORE // QBLK        # 4
P = 128
KSUB = HID // P                # 8
MSUB = HID // P                # 8
JTOT = NNB * L                 # 256 keys per chunk
SCALE = 0.125                  # 1/sqrt(HEAD)

_CACHE = {}


def _build():
    from contextlib import ExitStack

    import concourse.bass as bass
    import concourse.mybir as mybir
    import concourse.tile as tile
    from concourse import bacc

    f32 = mybir.dt.float32
    bf16 = mybir.dt.bfloat16

    nc = bacc.Bacc("TRN2", target_bir_lowering=False, debug=False,
                   num_devices=NCORES)

    q_t = nc.dram_tensor("q_t", [HID, PER_CORE * CHUNK], bf16, kind="ExternalInput")
    kv_t = nc.dram_tensor("kv_t", [HID, PER_CORE * JTOT], bf16, kind="ExternalInput")
    wq_t = nc.dram_tensor("wq_t", [MSUB * P * KSUB * P], bf16, kind="ExternalInput")
    wk_t = nc.dram_tensor("wk_t", [HID, HID], bf16, kind="ExternalInput")
    wv_t = nc.dram_tensor("wv_t", [HID, HID], bf16, kind="ExternalInput")
    bq_t = nc.dram_tensor("bq_t", [P, MSUB], f32, kind="ExternalInput")
    bk_t = nc.dram_tensor("bk_t", [P, MSUB], f32, kind="ExternalInput")
    bv_r = nc.dram_tensor("bv_r", [P, HID], f32, kind="ExternalInput")
    out = nc.dram_tensor("out", [PER_CORE, P, NHEADS * HEAD // 2], f32,
                         kind="ExternalOutput")

    q_td = q_t[:].rearrange("(ko p) r -> p ko r", p=P)
    kv_td = kv_t[:].rearrange("(ko p) r -> p ko r", p=P)
    wq_packed = wq_t[:].rearrange("(mo p ko m) -> mo p ko m",
                                  mo=MSUB, p=P, ko=KSUB)
    wk_td = wk_t[:].rearrange("(ko p) m -> p ko m", p=P)
    wv_td = wv_t[:].rearrange("(ko p) m -> p ko m", p=P)

    ROWS = QBLK * JTOT             # 1024 kv rows per block
    GRP = ROWS // 2                # 512 rows per K-projection group
    QR = QBLK * CHUNK              # 256 q rows per block

    with tile.TileContext(nc) as tc:
        with ExitStack() as ctx:
            wpool = ctx.enter_context(tc.tile_pool(name="weights", bufs=1))
            qtp = ctx.enter_context(tc.tile_pool(name="qt", bufs=2))
            kvp = ctx.enter_context(tc.tile_pool(name="kvt", bufs=4))
            kpp = ctx.enter_context(tc.tile_pool(name="kproj", bufs=2))
            sfp = ctx.enter_context(tc.tile_pool(name="soft", bufs=16))
            smalls = ctx.enter_context(tc.tile_pool(name="smalls", bufs=6))
            outp = ctx.enter_context(tc.tile_pool(name="outsb", bufs=2))
            ps_pj = ctx.enter_context(tc.tile_pool(name="ps_pj", bufs=3, space="PSUM"))
            ps_sc = ctx.enter_context(tc.tile_pool(name="ps_sc", bufs=2, space="PSUM"))
            ps_av = ctx.enter_context(tc.tile_pool(name="ps_av", bufs=2, space="PSUM"))

            # --- resident constants, ordered so the first Q matmul can
            # start ASAP and wk/kv arrive under the Q projection. ---
            bq_sb = wpool.tile([P, MSUB], f32)
            nc.sync.dma_start(bq_sb[:], bq_t[:])
            bk_sb = wpool.tile([P, MSUB], f32)
            nc.sync.dma_start(bk_sb[:], bk_t[:])
            wq_ts = []
            for mo in range(MSUB):
                w = wpool.tile([P, KSUB, P], bf16, name=f"wq{mo}")
                nc.sync.dma_start(w[:], wq_packed[mo])
                wq_ts.append(w)
                if mo == 0:
                    qt_first = qtp.tile([P, KSUB, QR], bf16, tag="qt",
                                        name="qt_sb")
                    nc.sync.dma_start(qt_first[:], q_td[:, :, bass.ts(0, QR)])
            # wk per m-tile so the first K matmul starts after 256KB
            wk_sb = wpool.tile([P, KSUB, HID], bf16)
            for mo in range(MSUB):
                nc.sync.dma_start(wk_sb[:, :, bass.ts(mo, P)],
                                  wk_td[:, :, bass.ts(mo, P)])
            kvt_first = []
            for g2 in range(2):
                kvt_sb = kvp.tile([P, KSUB, GRP], bf16, tag="kvt",
                                  name="kvt_sb")
                nc.sync.dma_start(kvt_sb[:],
                                  kv_td[:, :, bass.ds(g2 * GRP, GRP)])
                kvt_first.append(kvt_sb)
            wv_sb = wpool.tile([P, KSUB, HID], bf16)
            for nt in range(2):
                nc.sync.dma_start(wv_sb[:, :, bass.ts(nt, 512)],
                                  wv_td[:, :, bass.ts(nt, 512)])
            bv_sb = wpool.tile([P, HID], f32)
            nc.sync.dma_start(bv_sb[:], bv_r[:])

            # persistent ping-pong tiles: block-diagonal q (off-diagonal
            # zeros written once here, diagonal rewritten per block) and
            # V-with-ones-column (col 128 written once).
            qpbd_ts = []
            vp_ts = []
            for i in range(2):
                qpbd = wpool.tile([P, QBLK, MSUB, P], bf16, name=f"qpbd{i}")
                nc.vector.memset(qpbd[:], 0.0)
                qpbd_ts.append(qpbd)
                vp = wpool.tile([P, 2 * QBLK, MSUB, P + 1], bf16,
                                name=f"vp{i}")
                nc.vector.memset(vp[:, :, :, P], 1.0)
                vp_ts.append(vp)

            Exp = mybir.ActivationFunctionType.Exp
            Ident = mybir.ActivationFunctionType.Identity

            for blk in range(NBLK):
                qpbd = qpbd_ts[blk % 2]
                vp = vp_ts[blk % 2]

                if blk == 0:
                    qt_sb = qt_first
                    kvt_ts = kvt_first
                else:
                    qt_sb = qtp.tile([P, KSUB, QR], bf16, tag="qt",
                                     name="qt_sb")
                    nc.sync.dma_start(qt_sb[:],
                                      q_td[:, :, bass.ts(blk, QR)])
                    kvt_ts = []
                    for g2 in range(2):
                        kvt_sb = kvp.tile([P, KSUB, GRP], bf16, tag="kvt",
                                          name="kvt_sb")
                        nc.sync.dma_start(
                            kvt_sb[:],
                            kv_td[:, :, bass.ds(blk * ROWS + g2 * GRP, GRP)])
                        kvt_ts.append(kvt_sb)

                # ---- Q projection -> block-diagonal qpbd ----
                for mo in range(MSUB):
                    pt = ps_pj.tile([P, 512], f32, tag="ps_pj", name="pt")
                    ptq = pt[:, :QR]
                    for k in range(KSUB):
                        nc.tensor.matmul(
                            ptq[:],
                            wq_ts[mo][:, k, :],
                            qt_sb[:, k, :],
                            start=(k == 0),
                            stop=(k == KSUB - 1),
                        )
                    for pi in range(QBLK):
                        nc.scalar.activation(
                            qpbd[0:64, pi, mo, 0:64],
                            pt[0:64, bass.ts(pi, CHUNK)], Ident,
                            bias=bq_sb[0:64, mo, None])
                        nc.scalar.activation(
                            qpbd[64:128, pi, mo, 64:128],
                            pt[64:128, bass.ts(pi, CHUNK)], Ident,
                            bias=bq_sb[64:128, mo, None])

                # ---- K projection (2 groups of 512 kv rows) ----
                kp_sb = kpp.tile([P, MSUB, ROWS], bf16, tag="kp", name="kp")
                for g2 in range(2):
                    for mo in range(MSUB):
                        pt = ps_pj.tile([P, 512], f32, tag="ps_pj", name="pt")
                        for k in range(KSUB):
                            nc.tensor.matmul(
                                pt[:],
                                wk_sb[:, k, bass.ts(mo, P)],
                                kvt_ts[g2][:, k, :],
                                start=(k == 0),
                                stop=(k == KSUB - 1),
                            )
                        nc.scalar.activation(
                            kp_sb[:, mo, bass.ts(g2, GRP)], pt[:],
                            Ident, bias=bk_sb[:, mo, None])

                # ---- V projection for one pair (512 rows = 2 row-tiles) ----
                def v_proj(pi):
                    for rt in (2 * pi, 2 * pi + 1):
                        g2, rl = rt // 4, rt % 4
                        for nt in range(2):
                            pt = ps_pj.tile([P, 512], f32, tag="ps_pj",
                                            name="pt")
                            for k in range(KSUB):
                                nc.tensor.matmul(
                                    pt[:],
                                    kvt_ts[g2][:, k, bass.ts(rl, P)],
                                    wv_sb[:, k, bass.ts(nt, 512)],
                                    start=(k == 0),
                                    stop=(k == KSUB - 1),
                                )
                            for j in range(4):
                                nc.vector.tensor_tensor(
                                    vp[:, rt, 4 * nt + j, 0:P],
                                    pt[:, bass.ts(j, P)],
                                    bv_sb[:, bass.ds(nt * 512 + j * P, P)],
                                    mybir.AluOpType.add,
                                )

                # ---- scores (pre-transposed, 2 heads per matmul) ----
                def scores(pi):
                    ps_list = []
                    for hp in range(MSUB):
                        ps_s = ps_sc.tile([P, 2, P], f32, tag="ps_s")
                        for jh in range(2):
                            nc.tensor.matmul(
                                ps_s[:, jh, :],
                                kp_sb[:, hp, bass.ds(pi * JTOT + jh * P, P)],
                                qpbd[:, pi, hp, :],
                                start=True, stop=True,
                            )
                        attn = sfp.tile([P, 2, P], bf16, tag="attn")
                        nc.scalar.activation(attn[:], ps_s[:], Exp,
                                             scale=SCALE)
                        ps_list.append(attn)
                    return ps_list

                # ---- AV + normalize + store for one pair ----
                def att_out(pi, attn_ts):
                    out_sb = outp.tile([P, MSUB, HEAD], f32, tag="out_sb")
                    for hp in range(MSUB):
                        attn = attn_ts[hp]
                        ps_o = ps_av.tile([P, P + 1], f32, tag="ps_o")
                        for jh in range(2):
                            nc.tensor.matmul(
                                ps_o[:],
                                attn[:, jh, :],
                                vp[:, 2 * pi + jh, hp, :],
                                start=(jh == 0), stop=(jh == 1),
                            )
                        recip = smalls.tile([P, 1], f32, tag="recip")
                        nc.vector.reciprocal(recip[:], ps_o[:, P, None])
                        nc.scalar.activation(
                            out_sb[0:64, hp, :], ps_o[0:64, 0:64],
                            Ident, scale=recip[0:64])
                        nc.scalar.activation(
                            out_sb[64:128, hp, :], ps_o[64:128, 64:128],
                            Ident, scale=recip[64:128])
                    nc.sync.dma_start(out[blk * QBLK + pi], out_sb[:])

                # interleave: V(0), S(0), [V(p+1), AV(p), S(p+1)]..., AV(3)
                v_proj(0)
                attn_p = scores(0)
                for pi in range(1, QBLK):
                    v_proj(pi)
                    att_out(pi - 1, attn_p)
                    attn_p = scores(pi)
                att_out(QBLK - 1, attn_p)

    nc.finalize()
    return nc


def _prepare_inputs(query, kv, Wq, bq, Wk, bk, Wv, bv):
    """Build the 8 per-core input maps (host-side shard + layout + cast)."""
    import ml_dtypes

    f32 = np.float32
    bf = ml_dtypes.bfloat16
    query = np.asarray(query, dtype=f32)
    kv = np.asarray(kv, dtype=f32)

    # shift right by CHUNK-1, pad to C*CHUNK rows
    q_shift = np.zeros((B, C * CHUNK, HID), dtype=f32)
    q_shift[:, : S - (CHUNK - 1)] = query[:, CHUNK - 1:]
    q_pairs = q_shift.reshape(B * C, CHUNK, HID)
    kv_pairs = kv.reshape(B * C, JTOT, HID)

    wq_tt = np.asarray(Wq, dtype=f32).T  # [h, m]
    wq_t = np.ascontiguousarray(
        wq_tt.reshape(KSUB, P, MSUB, P).transpose(2, 1, 0, 3)
    ).reshape(-1).astype(bf)
    wk_t = np.asarray(Wk, dtype=f32).T.astype(bf)
    wv_t = np.asarray(Wv, dtype=f32).T.astype(bf)
    bq_t = np.ascontiguousarray(np.asarray(bq, dtype=f32).reshape(MSUB, P).T)
    bk_t = np.ascontiguousarray(np.asarray(bk, dtype=f32).reshape(MSUB, P).T)
    bv_rep = np.ascontiguousarray(
        np.broadcast_to(np.asarray(bv, dtype=f32), (P, HID)))

    in_maps = []
    for ci in range(NCORES):
        sel = slice(ci * PER_CORE, (ci + 1) * PER_CORE)
        q_core = q_pairs[sel].reshape(PER_CORE * CHUNK, HID)
        kv_core = kv_pairs[sel].reshape(PER_CORE * JTOT, HID)
        in_maps.append({
            "q_t": np.ascontiguousarray(q_core.T.astype(bf)),
            "kv_t": np.ascontiguousarray(kv_core.T.astype(bf)),
            "wq_t": wq_t,
            "wk_t": np.ascontiguousarray(wk_t),
            "wv_t": np.ascontiguousarray(wv_t),
            "bq_t": bq_t,
            "bk_t": bk_t,
            "bv_r": bv_rep,
        })
    return in_maps


def _unpack_output(results):
    """results: list of 8 dicts with 'out' [16, 128, 512] -> full (B,S,HID)."""
    h = np.empty((NPAIRS, CHUNK, HID), dtype=np.float32)
    for ci in range(NCORES):
        arr = results[ci]["out"]
        a = arr.reshape(PER_CORE, 2, CHUNK, NHEADS // 2, HEAD)
        a = a.transpose(0, 2, 3, 1, 4).reshape(PER_CORE, CHUNK, HID)
        h[ci * PER_CORE:(ci + 1) * PER_CORE] = a
    h = h.reshape(B, C * CHUNK, HID)
    outp = np.zeros((B, S, HID), dtype=np.float32)
    outp[:, CHUNK - 1:] = h[:, : S - (CHUNK - 1)]
    return outp


def kernel(query, kv, Wq, bq, Wk, bk, Wv, bv):
    from concourse.bass_utils import run_bass_kernel_spmd

    if "nc" not in _CACHE:
        _CACHE["nc"] = _build()
    nc = _CACHE["nc"]

    in_maps = _prepare_inputs(query, kv, Wq, bq, Wk, bk, Wv, bv)
    res = run_bass_kernel_spmd(nc, in_maps, list(range(NCORES)))
    return _unpack_output(res.results)


# revision 11
# speedup vs baseline: 1.2546x; 1.2166x over previous
"""Chunked cross-attention TRN2 kernel (8 NeuronCores, SPMD).

Problem (hardcoded): B=4, S=2048, HIDDEN=1024, heads=16, head_dim=64,
chunks C=32 x chunk_len 64, neighbors N=2 x L=128 (256 keys per chunk).

Sharding: the B*C = 128 (batch, chunk) pairs are split evenly across the 8
cores (16 pairs each). Each core projects Q/K/V for its pairs, runs the
chunk-local softmax attention, and writes its pairs' outputs. Weights are
replicated per core. No collectives needed.

All matmul operands are bf16 (stationary tiles are 128-column so FWL is
active; fp32 accumulation in PSUM). The attention avoids PE transposes
entirely: scores are computed pre-transposed as out[keys, queries] with a
block-diagonal [dA|dB]x[qA|qB] moving operand built during the Q
projection (zeros kill the cross-head terms), packing two heads per
matmul. The AV matmul consumes the [keys, q] attention weights directly;
a ones-column appended to V yields the softmax normalizer in the same
matmul, and the 1/sum scale is applied by ScalarE during PSUM copyback.

Per-block schedule interleaves the V projection with attention per pair
so the ScalarE exp runs under PE matmuls of the next pair.
"""

import numpy as np

B, S, HID = 4, 2048, 1024
C, NNB, L = 32, 2, 128
CHUNK = 64
NHEADS, HEAD = 16, 64
NCORES = 8
NPAIRS = B * C                 # 128
PER_CORE = NPAIRS // NCORES    # 16
QBLK = 4                       # pairs per block (256 q rows, 1024 kv rows)
NBLK = PER_CORE // QBLK        # 4
P = 128
KSUB = HID // P                # 8
MSUB = HID // P                # 8
JTOT = NNB * L                 # 256 keys per chunk
SCALE = 0.125                  # 1/sqrt(HEAD)

_CACHE = {}


def _build():
    from contextlib import ExitStack

    import concourse.bass as bass
    import concourse.mybir as mybir
    import concourse.tile as tile
    from concourse import bacc

    f32 = mybir.dt.float32
    bf16 = mybir.dt.bfloat16

    nc = bacc.Bacc("TRN2", target_bir_lowering=False, debug=False,
                   num_devices=NCORES)

    q_t = nc.dram_tensor("q_t", [HID, PER_CORE * CHUNK], bf16, kind="ExternalInput")
    kv_t = nc.dram_tensor("kv_t", [HID, PER_CORE * JTOT], bf16, kind="ExternalInput")
    wq_t = nc.dram_tensor("wq_t", [MSUB * P * KSUB * P], bf16, kind="ExternalInput")
    wk_t = nc.dram_tensor("wk_t", [HID, HID], bf16, kind="ExternalInput")
    wv_t = nc.dram_tensor("wv_t", [HID, HID], bf16, kind="ExternalInput")
    bq_t = nc.dram_tensor("bq_t", [P, MSUB], f32, kind="ExternalInput")
    bk_t = nc.dram_tensor("bk_t", [P, MSUB], f32, kind="ExternalInput")
    bv_r = nc.dram_tensor("bv_r", [P, HID], f32, kind="ExternalInput")
    out = nc.dram_tensor("out", [PER_CORE, P, NHEADS * HEAD // 2], f32,
                         kind="ExternalOutput")

    q_td = q_t[:].rearrange("(ko p) r -> p ko r", p=P)
    kv_td = kv_t[:].rearrange("(ko p) r -> p ko r", p=P)
    wq_packed = wq_t[:].rearrange("(mo p ko m) -> mo p ko m",
                                  mo=MSUB, p=P, ko=KSUB)
    wk_td = wk_t[:].rearrange("(ko p) m -> p ko m", p=P)
    wv_td = wv_t[:].rearrange("(ko p) m -> p ko m", p=P)

    ROWS = QBLK * JTOT             # 1024 kv rows per block
    GRP = ROWS // 2                # 512 rows per K-projection group
    QR = QBLK * CHUNK              # 256 q rows per block

    with tile.TileContext(nc) as tc:
        with ExitStack() as ctx:
            wpool = ctx.enter_context(tc.tile_pool(name="weights", bufs=1))
            qtp = ctx.enter_context(tc.tile_pool(name="qt", bufs=2))
            kvp = ctx.enter_context(tc.tile_pool(name="kvt", bufs=4))
            kpp = ctx.enter_context(tc.tile_pool(name="kproj", bufs=2))
            sfp = ctx.enter_context(tc.tile_pool(name="soft", bufs=16))
            smalls = ctx.enter_context(tc.tile_pool(name="smalls", bufs=6))
            outp = ctx.enter_context(tc.tile_pool(name="outsb", bufs=2))
            ps_pj = ctx.enter_context(tc.tile_pool(name="ps_pj", bufs=4, space="PSUM"))
            ps_sc = ctx.enter_context(tc.tile_pool(name="ps_sc", bufs=2, space="PSUM"))
            ps_av = ctx.enter_context(tc.tile_pool(name="ps_av", bufs=2, space="PSUM"))

            # --- resident constants, ordered so the first Q matmul can
            # start ASAP and wk/kv arrive under the Q projection. ---
            wq_ts = []
            for mo in range(MSUB):
                w = wpool.tile([P, KSUB, P], bf16, name=f"wq{mo}")
                nc.sync.dma_start(w[:], wq_packed[mo])
                wq_ts.append(w)
                if mo == 0:
                    qt_first = qtp.tile([P, KSUB, QR], bf16, tag="qt",
                                        name="qt_sb")
                    nc.sync.dma_start(qt_first[:], q_td[:, :, bass.ts(0, QR)])
            bq_sb = wpool.tile([P, MSUB], f32)
            nc.sync.dma_start(bq_sb[:], bq_t[:])
            bk_sb = wpool.tile([P, MSUB], f32)
            nc.sync.dma_start(bk_sb[:], bk_t[:])
            # wk per m-tile so the first K matmul starts after 256KB
            wk_sb = wpool.tile([P, KSUB, HID], bf16)
            for mo in range(MSUB):
                nc.sync.dma_start(wk_sb[:, :, bass.ts(mo, P)],
                                  wk_td[:, :, bass.ts(mo, P)])
            kvt_first = []
            for g2 in range(2):
                kvt_sb = kvp.tile([P, KSUB, GRP], bf16, tag="kvt",
                                  name="kvt_sb")
                nc.sync.dma_start(kvt_sb[:],
                                  kv_td[:, :, bass.ds(g2 * GRP, GRP)])
                kvt_first.append(kvt_sb)
            wv_sb = wpool.tile([P, KSUB, HID], bf16)
            for nt in range(2):
                nc.sync.dma_start(wv_sb[:, :, bass.ts(nt, 512)],
                                  wv_td[:, :, bass.ts(nt, 512)])
            bv_sb = wpool.tile([P, HID], f32)
            nc.sync.dma_start(bv_sb[:], bv_r[:])

            # persistent ping-pong tiles: block-diagonal q (off-diagonal
            # zeros written once here, diagonal rewritten per block) and
            # V-with-ones-column (col 128 written once).
            qpbd_ts = []
            vp_ts = []
            for i in range(2):
                qpbd = wpool.tile([P, QBLK, MSUB, P], bf16, name=f"qpbd{i}")
                nc.vector.memset(qpbd[:], 0.0)
                qpbd_ts.append(qpbd)
                vp = wpool.tile([P, 2 * QBLK, MSUB, P + 1], bf16,
                                name=f"vp{i}")
                nc.vector.memset(vp[:, :, :, P], 1.0)
                vp_ts.append(vp)

            Exp = mybir.ActivationFunctionType.Exp
            Ident = mybir.ActivationFunctionType.Identity

            for blk in range(NBLK):
                qpbd = qpbd_ts[blk % 2]
                vp = vp_ts[blk % 2]

                if blk == 0:
                    qt_sb = qt_first
                    kvt_ts = kvt_first
                else:
                    qt_sb = qtp.tile([P, KSUB, QR], bf16, tag="qt",
                                     name="qt_sb")
                    nc.sync.dma_start(qt_sb[:],
                                      q_td[:, :, bass.ts(blk, QR)])
                    kvt_ts = []
                    for g2 in range(2):
                        kvt_sb = kvp.tile([P, KSUB, GRP], bf16, tag="kvt",
                                          name="kvt_sb")
                        nc.sync.dma_start(
                            kvt_sb[:],
                            kv_td[:, :, bass.ds(blk * ROWS + g2 * GRP, GRP)])
                        kvt_ts.append(kvt_sb)

                # ---- Q projection -> block-diagonal qpbd ----
                for mo in range(MSUB):
                    pt = ps_pj.tile([P, 512], f32, tag="ps_pj", name="pt")
                    ptq = pt[:, :QR]
                    for k in range(KSUB):
                        nc.tensor.matmul(
                            ptq[:],
                            wq_ts[mo][:, k, :],
                            qt_sb[:, k, :],
                            start=(k == 0),
                            stop=(k == KSUB - 1),
                        )
                    nc.scalar.activation(
                        qpbd[0:64, :, mo, 0:64], ptq[0:64, :], Ident,
                        bias=bq_sb[0:64, mo, None])
                    nc.scalar.activation(
                        qpbd[64:128, :, mo, 64:128], ptq[64:128, :], Ident,
                        bias=bq_sb[64:128, mo, None])

                # ---- K projection (2 groups of 512 kv rows) ----
                kp_sb = kpp.tile([P, MSUB, ROWS], bf16, tag="kp", name="kp")
                for g2 in range(2):
                    for mo in range(MSUB):
                        pt = ps_pj.tile([P, 512], f32, tag="ps_pj", name="pt")
                        for k in range(KSUB):
                            nc.tensor.matmul(
                                pt[:],
                                wk_sb[:, k, bass.ts(mo, P)],
                                kvt_ts[g2][:, k, :],
                                start=(k == 0),
                                stop=(k == KSUB - 1),
                            )
                        nc.scalar.activation(
                            kp_sb[:, mo, bass.ts(g2, GRP)], pt[:],
                            Ident, bias=bk_sb[:, mo, None])

                # ---- V projection for one pair (512 rows = 2 row-tiles) ----
                def v_proj(pi):
                    for rt in (2 * pi, 2 * pi + 1):
                        g2, rl = rt // 4, rt % 4
                        for nt in range(2):
                            pt = ps_pj.tile([P, 512], f32, tag="ps_pj",
                                            name="pt")
                            for k in range(KSUB):
                                nc.tensor.matmul(
                                    pt[:],
                                    kvt_ts[g2][:, k, bass.ts(rl, P)],
                                    wv_sb[:, k, bass.ts(nt, 512)],
                                    start=(k == 0),
                                    stop=(k == KSUB - 1),
                                )
                            nc.vector.tensor_tensor(
                                vp[:, rt, 4 * nt:4 * nt + 4, 0:P],
                                pt[:],
                                bv_sb[:, bass.ts(nt, 512)],
                                mybir.AluOpType.add,
                            )

                # ---- scores (pre-transposed, 2 heads per matmul) ----
                def scores(pi):
                    ps_list = []
                    for hp in range(MSUB):
                        ps_s = ps_sc.tile([P, 2, P], f32, tag="ps_s")
                        for jh in range(2):
                            nc.tensor.matmul(
                                ps_s[:, jh, :],
                                kp_sb[:, hp, bass.ds(pi * JTOT + jh * P, P)],
                                qpbd[:, pi, hp, :],
                                start=True, stop=True,
                            )
                        attn = sfp.tile([P, 2, P], bf16, tag="attn")
                        nc.scalar.activation(attn[:], ps_s[:], Exp,
                                             scale=SCALE)
                        ps_list.append(attn)
                    return ps_list

                # ---- AV + normalize + store for one pair ----
                # (vp/base bound at def time: the last pair's call is
                # deferred into the next block iteration)
                def att_out(pi, attn_ts, vp=vp, base=blk * QBLK):
                    out_sb = outp.tile([P, MSUB, HEAD], f32, tag="out_sb")
                    for hp in range(MSUB):
                        attn = attn_ts[hp]
                        ps_o = ps_av.tile([P, P + 1], f32, tag="ps_o")
                        for jh in range(2):
                            nc.tensor.matmul(
                                ps_o[:],
                                attn[:, jh, :],
                                vp[:, 2 * pi + jh, hp, :],
                                start=(jh == 0), stop=(jh == 1),
                            )
                        recip = smalls.tile([P, 1], f32, tag="recip")
                        nc.vector.reciprocal(recip[:], ps_o[:, P, None])
                        nc.vector.tensor_scalar_mul(
                            out_sb[0:64, hp, :], ps_o[0:64, 0:64],
                            recip[0:64])
                        nc.vector.tensor_scalar_mul(
                            out_sb[64:128, hp, :], ps_o[64:128, 64:128],
                            recip[64:128])
                    nc.sync.dma_start(out[base + pi], out_sb[:])

                # finish the previous block's last pair now that this
                # block's Q/K projections have covered its exp latency
                if blk > 0:
                    prev_att_out(QBLK - 1, prev_attn)

                # interleave: V(0), S(0), [V(p+1), AV(p), S(p+1)]...; the
                # last pair's AV is deferred into the next block
                v_proj(0)
                attn_p = scores(0)
                for pi in range(1, QBLK):
                    v_proj(pi)
                    att_out(pi - 1, attn_p)
                    attn_p = scores(pi)
                prev_att_out = att_out
                prev_attn = attn_p

            prev_att_out(QBLK - 1, prev_attn)

    nc.finalize()
    return nc


def _prepare_inputs(query, kv, Wq, bq, Wk, bk, Wv, bv):
    """Build the 8 per-core input maps (host-side shard + layout + cast)."""
    import ml_dtypes

    f32 = np.float32
    bf = ml_dtypes.bfloat16
    query = np.asarray(query, dtype=f32)
    kv = np.asarray(kv, dtype=f32)

    # shift right by CHUNK-1, pad to C*CHUNK rows
    q_shift = np.zeros((B, C * CHUNK, HID), dtype=f32)
    q_shift[:, : S - (CHUNK - 1)] = query[:, CHUNK - 1:]
    q_pairs = q_shift.reshape(B * C, CHUNK, HID)
    kv_pairs = kv.reshape(B * C, JTOT, HID)

    wq_tt = np.asarray(Wq, dtype=f32).T  # [h, m]
    wq_t = np.ascontiguousarray(
        wq_tt.reshape(KSUB, P, MSUB, P).transpose(2, 1, 0, 3)
    ).reshape(-1).astype(bf)
    wk_t = np.asarray(Wk, dtype=f32).T.astype(bf)
    wv_t = np.asarray(Wv, dtype=f32).T.astype(bf)
    bq_t = np.ascontiguousarray(np.asarray(bq, dtype=f32).reshape(MSUB, P).T)
    bk_t = np.ascontiguousarray(np.asarray(bk, dtype=f32).reshape(MSUB, P).T)
    bv_rep = np.ascontiguousarray(
        np.broadcast_to(np.asarray(bv, dtype=f32), (P, HID)))

    in_maps = []
    for ci in range(NCORES):
        sel = slice(ci * PER_CORE, (ci + 1) * PER_CORE)
        q_core = q_pairs[sel].reshape(PER_CORE * CHUNK, HID)
        kv_core = kv_pairs[sel].reshape(PER_CORE * JTOT, HID)
        in_maps.append({
            "q_t": np.ascontiguousarray(q_core.T.astype(bf)),
            "kv_t": np.ascontiguousarray(kv_core.T.astype(bf)),
            "wq_t": wq_t,
            "wk_t": np.ascontiguousarray(wk_t),
            "wv_t": np.ascontiguousarray(wv_t),
            "bq_t": bq_t,
            "bk_t": bk_t,
            "bv_r": bv_rep,
        })
    return in_maps


def _unpack_output(results):
    """results: list of 8 dicts with 'out' [16, 128, 512] -> full (B,S,HID)."""
    h = np.empty((NPAIRS, CHUNK, HID), dtype=np.float32)
    for ci in range(NCORES):
        arr = results[ci]["out"]
        a = arr.reshape(PER_CORE, 2, CHUNK, NHEADS // 2, HEAD)
        a = a.transpose(0, 2, 3, 1, 4).reshape(PER_CORE, CHUNK, HID)
        h[ci * PER_CORE:(ci + 1) * PER_CORE] = a
    h = h.reshape(B, C * CHUNK, HID)
    outp = np.zeros((B, S, HID), dtype=np.float32)
    outp[:, CHUNK - 1:] = h[:, : S - (CHUNK - 1)]
    return outp


def kernel(query, kv, Wq, bq, Wk, bk, Wv, bv):
    from concourse.bass_utils import run_bass_kernel_spmd

    if "nc" not in _CACHE:
        _CACHE["nc"] = _build()
    nc = _CACHE["nc"]

    in_maps = _prepare_inputs(query, kv, Wq, bq, Wk, bk, Wv, bv)
    res = run_bass_kernel_spmd(nc, in_maps, list(range(NCORES)))
    return _unpack_output(res.results)


# revision 21
# speedup vs baseline: 1.2835x; 1.0230x over previous
"""Chunked cross-attention TRN2 kernel (8 NeuronCores, SPMD).

Problem (hardcoded): B=4, S=2048, HIDDEN=1024, heads=16, head_dim=64,
chunks C=32 x chunk_len 64, neighbors N=2 x L=128 (256 keys per chunk).

Sharding: the B*C = 128 (batch, chunk) pairs are split evenly across the 8
cores (16 pairs each). Each core projects Q/K/V for its pairs, runs the
chunk-local softmax attention, and writes its pairs' outputs. Weights are
replicated per core. No collectives needed.

All matmul operands are bf16 (stationary tiles are 128-column so FWL is
active; fp32 accumulation in PSUM). The attention avoids PE transposes
entirely: scores are computed pre-transposed as out[keys, queries] with a
block-diagonal [dA|dB]x[qA|qB] moving operand built during the Q
projection (zeros kill the cross-head terms), packing two heads per
matmul. The AV matmul consumes the [keys, q] attention weights directly;
a ones-column appended to V yields the softmax normalizer in the same
matmul, and the 1/sum scale is applied by ScalarE during PSUM copyback.

Per-block schedule interleaves the V projection with attention per pair
so the ScalarE exp runs under PE matmuls of the next pair.
"""

import numpy as np

B, S, HID = 4, 2048, 1024
C, NNB, L = 32, 2, 128
CHUNK = 64
NHEADS, HEAD = 16, 64
NCORES = 8
NPAIRS = B * C                 # 128
PER_CORE = NPAIRS // NCORES    # 16
QBLK = 4                       # pairs per block (256 q rows, 1024 kv rows)
NBLK = PER_CORE // QBLK        # 4
P = 128
KSUB = HID // P                # 8
MSUB = HID // P                # 8
JTOT = NNB * L                 # 256 keys per chunk
SCALE = 0.125                  # 1/sqrt(HEAD)

_CACHE = {}


def _build():
    from contextlib import ExitStack

    import concourse.bass as bass
    import concourse.mybir as mybir
    import concourse.tile as tile
    from concourse import bacc

    f32 = mybir.dt.float32
    bf16 = mybir.dt.bfloat16

    nc = bacc.Bacc("TRN2", target_bir_lowering=False, debug=False,
                   num_devices=NCORES)

    q_t = nc.dram_tensor("q_t", [HID, PER_CORE * CHUNK], bf16, kind="ExternalInput")
    kv_t = nc.dram_tensor("kv_t", [HID, PER_CORE * JTOT], bf16, kind="ExternalInput")
    wq_t = nc.dram_tensor("wq_t", [MSUB * P * KSUB * P], bf16, kind="ExternalInput")
    wk_t = nc.dram_tensor("wk_t", [HID, HID], bf16, kind="ExternalInput")
    wv_t = nc.dram_tensor("wv_t", [HID, HID], bf16, kind="ExternalInput")
    bqk_t = nc.dram_tensor("bqk_t", [P, 2 * MSUB], f32, kind="ExternalInput")
    bv_r = nc.dram_tensor("bv_r", [P, HID], f32, kind="ExternalInput")
    out = nc.dram_tensor("out", [PER_CORE, P, NHEADS * HEAD // 2], f32,
                         kind="ExternalOutput")

    q_td = q_t[:].rearrange("(ko p) r -> p ko r", p=P)
    kv_td = kv_t[:].rearrange("(ko p) r -> p ko r", p=P)
    wq_packed = wq_t[:].rearrange("(mo p ko m) -> p mo ko m",
                                  mo=MSUB, p=P, ko=KSUB)
    wk_td = wk_t[:].rearrange("(ko p) m -> p ko m", p=P)
    wv_td = wv_t[:].rearrange("(ko p) m -> p ko m", p=P)

    ROWS = QBLK * JTOT             # 1024 kv rows per block
    GRP = ROWS // 2                # 512 rows per K-projection group
    QR = QBLK * CHUNK              # 256 q rows per block

    with tile.TileContext(nc) as tc:
        with ExitStack() as ctx:
            wpool = ctx.enter_context(tc.tile_pool(name="weights", bufs=1))
            qtp = ctx.enter_context(tc.tile_pool(name="qt", bufs=2))
            kvp = ctx.enter_context(tc.tile_pool(name="kvt", bufs=4))
            kpp = ctx.enter_context(tc.tile_pool(name="kproj", bufs=2))
            sfp = ctx.enter_context(tc.tile_pool(name="soft", bufs=24))
            smalls = ctx.enter_context(tc.tile_pool(name="smalls", bufs=6))
            outp = ctx.enter_context(tc.tile_pool(name="outsb", bufs=2))
            ps_pj = ctx.enter_context(tc.tile_pool(name="ps_pj", bufs=4, space="PSUM"))
            ps_sc = ctx.enter_context(tc.tile_pool(name="ps_sc", bufs=2, space="PSUM"))
            ps_av = ctx.enter_context(tc.tile_pool(name="ps_av", bufs=2, space="PSUM"))

            # persistent ping-pong tiles: block-diagonal q (off-diagonal
            # zeros written once here, diagonal rewritten per block) and
            # V-with-ones-column (col 128 written once).
            qpbd_ts = []
            vp_ts = []
            for i in range(2):
                qpbd = wpool.tile([P, QBLK, MSUB, P], bf16, name=f"qpbd{i}")
                nc.vector.memset(qpbd[:], 0.0)
                qpbd_ts.append(qpbd)
                vp = wpool.tile([P, 2 * QBLK, MSUB, P + 1], bf16,
                                name=f"vp{i}")
                nc.vector.memset(vp[:, :, :, P], 1.0)
                vp_ts.append(vp)

            # --- resident constants; one dma_start each (descriptor
            # generation on the sync queue costs ~1us per dma_start) in
            # the order compute first needs them. ---
            wq_sb = wpool.tile([P, MSUB, KSUB, P], bf16)
            nc.sync.dma_start(wq_sb[:], wq_packed)
            qt_first = qtp.tile([P, KSUB, QR], bf16, tag="qt", name="qt_sb")
            nc.sync.dma_start(qt_first[:], q_td[:, :, bass.ts(0, QR)])
            bqk_sb = wpool.tile([P, 2 * MSUB], f32)
            nc.sync.dma_start(bqk_sb[:], bqk_t[:])
            wk_sb = wpool.tile([P, KSUB, HID], bf16)
            nc.sync.dma_start(wk_sb[:], wk_td)
            kvt_first = []
            for g2 in range(2):
                kvt_sb = kvp.tile([P, KSUB, GRP], bf16, tag="kvt",
                                  name="kvt_sb")
                nc.sync.dma_start(kvt_sb[:],
                                  kv_td[:, :, bass.ds(g2 * GRP, GRP)])
                kvt_first.append(kvt_sb)
            wv_sb = wpool.tile([P, KSUB, HID], bf16)
            nc.sync.dma_start(wv_sb[:], wv_td)
            bv_sb = wpool.tile([P, HID], f32)
            nc.sync.dma_start(bv_sb[:], bv_r[:])

            Exp = mybir.ActivationFunctionType.Exp
            Ident = mybir.ActivationFunctionType.Identity

            for blk in range(NBLK):
                qpbd = qpbd_ts[blk % 2]
                vp = vp_ts[blk % 2]

                if blk == 0:
                    qt_sb = qt_first
                    kvt_ts = kvt_first
                else:
                    qt_sb = qtp.tile([P, KSUB, QR], bf16, tag="qt",
                                     name="qt_sb")
                    nc.sync.dma_start(qt_sb[:],
                                      q_td[:, :, bass.ts(blk, QR)])
                    kvt_ts = []
                    for g2 in range(2):
                        kvt_sb = kvp.tile([P, KSUB, GRP], bf16, tag="kvt",
                                          name="kvt_sb")
                        nc.sync.dma_start(
                            kvt_sb[:],
                            kv_td[:, :, bass.ds(blk * ROWS + g2 * GRP, GRP)])
                        kvt_ts.append(kvt_sb)

                # ---- Q projection -> block-diagonal qpbd ----
                for mo in range(MSUB):
                    pt = ps_pj.tile([P, 512], f32, tag="ps_pj", name="pt")
                    ptq = pt[:, :QR]
                    for k in range(KSUB):
                        nc.tensor.matmul(
                            ptq[:],
                            wq_sb[:, mo, k, :],
                            qt_sb[:, k, :],
                            start=(k == 0),
                            stop=(k == KSUB - 1),
                        )
                    nc.scalar.activation(
                        qpbd[0:64, :, mo, 0:64], ptq[0:64, :], Ident,
                        bias=bqk_sb[0:64, mo, None])
                    nc.scalar.activation(
                        qpbd[64:128, :, mo, 64:128], ptq[64:128, :], Ident,
                        bias=bqk_sb[64:128, mo, None])

                # ---- K projection (2 groups of 512 kv rows) ----
                kp_sb = kpp.tile([P, MSUB, ROWS], bf16, tag="kp", name="kp")
                for g2 in range(2):
                    for mo in range(MSUB):
                        pt = ps_pj.tile([P, 512], f32, tag="ps_pj", name="pt")
                        for k in range(KSUB):
                            nc.tensor.matmul(
                                pt[:],
                                wk_sb[:, k, bass.ts(mo, P)],
                                kvt_ts[g2][:, k, :],
                                start=(k == 0),
                                stop=(k == KSUB - 1),
                            )
                        nc.scalar.activation(
                            kp_sb[:, mo, bass.ts(g2, GRP)], pt[:],
                            Ident, bias=bqk_sb[:, MSUB + mo, None])

                # ---- V projection for one pair (512 rows = 2 row-tiles) ----
                def v_proj(pi):
                    for rt in (2 * pi, 2 * pi + 1):
                        g2, rl = rt // 4, rt % 4
                        for nt in range(2):
                            pt = ps_pj.tile([P, 512], f32, tag="ps_pj",
                                            name="pt")
                            for k in range(KSUB):
                                nc.tensor.matmul(
                                    pt[:],
                                    kvt_ts[g2][:, k, bass.ts(rl, P)],
                                    wv_sb[:, k, bass.ts(nt, 512)],
                                    start=(k == 0),
                                    stop=(k == KSUB - 1),
                                )
                            nc.vector.tensor_tensor(
                                vp[:, rt, 4 * nt:4 * nt + 4, 0:P],
                                pt[:],
                                bv_sb[:, bass.ts(nt, 512)],
                                mybir.AluOpType.add,
                            )

                # ---- scores (pre-transposed, 2 heads per matmul) ----
                def scores(pi):
                    ps_list = []
                    for hp in range(MSUB):
                        ps_s = ps_sc.tile([P, 2, P], f32, tag="ps_s")
                        for jh in range(2):
                            nc.tensor.matmul(
                                ps_s[:, jh, :],
                                kp_sb[:, hp, bass.ds(pi * JTOT + jh * P, P)],
                                qpbd[:, pi, hp, :],
                                start=True, stop=True,
                            )
                        attn = sfp.tile([P, 2, P], bf16, tag="attn")
                        nc.scalar.activation(attn[:], ps_s[:], Exp,
                                             scale=SCALE)
                        ps_list.append(attn)
                    return ps_list

                # ---- AV + normalize + store for one pair ----
                # (vp/base bound at def time: the last pair's call is
                # deferred into the next block iteration)
                def att_out(pi, attn_ts, vp=vp, base=blk * QBLK):
                    out_sb = outp.tile([P, MSUB, HEAD], f32, tag="out_sb")
                    for hp in range(MSUB):
                        attn = attn_ts[hp]
                        ps_o = ps_av.tile([P, P + 1], f32, tag="ps_o")
                        for jh in range(2):
                            nc.tensor.matmul(
                                ps_o[:],
                                attn[:, jh, :],
                                vp[:, 2 * pi + jh, hp, :],
                                start=(jh == 0), stop=(jh == 1),
                            )
                        recip = smalls.tile([P, 1], f32, tag="recip")
                        nc.vector.reciprocal(recip[:], ps_o[:, P, None])
                        # normalize split across ScalarE/VectorE to keep
                        # both under the PE time of the attention phase
                        nc.scalar.activation(
                            out_sb[0:64, hp, :], ps_o[0:64, 0:64],
                            Ident, scale=recip[0:64])
                        nc.vector.tensor_scalar_mul(
                            out_sb[64:128, hp, :], ps_o[64:128, 64:128],
                            recip[64:128])
                    nc.sync.dma_start(out[base + pi], out_sb[:])

                # finish the previous block's last pair now that this
                # block's Q/K projections have covered its exp latency
                if blk > 0:
                    prev_att_out(QBLK - 1, prev_attn)

                # interleave: S(p) precedes V(p) so the exp chain hides
                # under the V projection; the last pair's AV is deferred
                # into the next block
                attn_p = scores(0)
                v_proj(0)
                attn_n = scores(1)
                v_proj(1)
                for pi in range(2, QBLK):
                    att_out(pi - 2, attn_p)
                    attn_p, attn_n = attn_n, scores(pi)
                    v_proj(pi)
                att_out(QBLK - 2, attn_p)
                prev_att_out = att_out
                prev_attn = attn_n

            prev_att_out(QBLK - 1, prev_attn)

    nc.finalize()
    return nc


def _prepare_inputs(query, kv, Wq, bq, Wk, bk, Wv, bv):
    """Build the 8 per-core input maps (host-side shard + layout + cast)."""
    import ml_dtypes

    f32 = np.float32
    bf = ml_dtypes.bfloat16
    query = np.asarray(query, dtype=f32)
    kv = np.asarray(kv, dtype=f32)

    # shift right by CHUNK-1, pad to C*CHUNK rows
    q_shift = np.zeros((B, C * CHUNK, HID), dtype=f32)
    q_shift[:, : S - (CHUNK - 1)] = query[:, CHUNK - 1:]
    q_pairs = q_shift.reshape(B * C, CHUNK, HID)
    kv_pairs = kv.reshape(B * C, JTOT, HID)

    wq_tt = np.asarray(Wq, dtype=f32).T  # [h, m]
    wq_t = np.ascontiguousarray(
        wq_tt.reshape(KSUB, P, MSUB, P).transpose(2, 1, 0, 3)
    ).reshape(-1).astype(bf)
    wk_t = np.asarray(Wk, dtype=f32).T.astype(bf)
    wv_t = np.asarray(Wv, dtype=f32).T.astype(bf)
    bqk_t = np.ascontiguousarray(np.concatenate([
        np.asarray(bq, dtype=f32).reshape(MSUB, P).T,
        np.asarray(bk, dtype=f32).reshape(MSUB, P).T], axis=1))
    bv_rep = np.ascontiguousarray(
        np.broadcast_to(np.asarray(bv, dtype=f32), (P, HID)))

    in_maps = []
    for ci in range(NCORES):
        sel = slice(ci * PER_CORE, (ci + 1) * PER_CORE)
        q_core = q_pairs[sel].reshape(PER_CORE * CHUNK, HID)
        kv_core = kv_pairs[sel].reshape(PER_CORE * JTOT, HID)
        in_maps.append({
            "q_t": np.ascontiguousarray(q_core.T.astype(bf)),
            "kv_t": np.ascontiguousarray(kv_core.T.astype(bf)),
            "wq_t": wq_t,
            "wk_t": np.ascontiguousarray(wk_t),
            "wv_t": np.ascontiguousarray(wv_t),
            "bqk_t": bqk_t,
            "bv_r": bv_rep,
        })
    return in_maps


def _unpack_output(results):
    """results: list of 8 dicts with 'out' [16, 128, 512] -> full (B,S,HID)."""
    h = np.empty((NPAIRS, CHUNK, HID), dtype=np.float32)
    for ci in range(NCORES):
        arr = results[ci]["out"]
        a = arr.reshape(PER_CORE, 2, CHUNK, NHEADS // 2, HEAD)
        a = a.transpose(0, 2, 3, 1, 4).reshape(PER_CORE, CHUNK, HID)
        h[ci * PER_CORE:(ci + 1) * PER_CORE] = a
    h = h.reshape(B, C * CHUNK, HID)
    outp = np.zeros((B, S, HID), dtype=np.float32)
    outp[:, CHUNK - 1:] = h[:, : S - (CHUNK - 1)]
    return outp


def kernel(query, kv, Wq, bq, Wk, bk, Wv, bv):
    from concourse.bass_utils import run_bass_kernel_spmd

    if "nc" not in _CACHE:
        _CACHE["nc"] = _build()
    nc = _CACHE["nc"]

    in_maps = _prepare_inputs(query, kv, Wq, bq, Wk, bk, Wv, bv)
    res = run_bass_kernel_spmd(nc, in_maps, list(range(NCORES)))
    return _unpack_output(res.results)
